# revision 13
# baseline (speedup 1.0000x reference)
"""Multi-head self-attention (B=2, T=2048, C=1024, H=16) on 8 trn2 NeuronCores.

Sharding: tensor-parallel over heads x data-parallel over batch.
Core c handles batch b = c//4 and the 4 heads [4g, 4g+4) where g = c%4.

Per-core device kernel (matmuls in float32r = full-rate single-pass fp32):
  A. qkT proj (transposed layout, K=C contraction, bias added on DVE):
     qkT tiles [128, 2048]: tile 0=[qT_h0;qT_h1] 1=[qT_h2;qT_h3]
                            2=[kT_h0;kT_h1] 3=[kT_h2;kT_h3]  (q pre-scaled 1/8)
     v proj in natural layout [token, d], bias via ones-row matmul, stored
     ones-augmented: v_aug[t, 65h+d], col 65h+64 = 1.0.
  B. attention, 128 flat steps (unit u = head-pair pr x q-tile qt, k-tile kt):
     S^T pair: two K=64 matmuls (array row-halves pack) into one 2-bank PSUM
     tile [128, 1024]; one wide exp on ScalarE -> E tile (SBUF, f32r).
     P.V runs LAG=4 steps behind: o_aug[65, 1024] accumulates v_aug.T @ E
     over 16 k-tiles (row 64 = softmax denominator, from the ones column).
     Finalize per unit: PE-transpose 128-col chunks, DVE reciprocal + scale
     -> natural-layout y tile. ScalarE does nothing but exp; all copies/bias
     on DVE so the exp table never reloads.
"""

import numpy as np

import concourse.bass as bass
import concourse.mybir as mybir
import concourse.tile as tile
from concourse import bacc, masks
from concourse.bass_utils import run_bass_kernel_spmd

N_CORES = 8
B, T, C = 2, 2048, 1024
D = 64            # head dim
HPC = 4           # heads per core
NT_C = C // 128   # 8 contraction tiles
NT_T = T // 128   # 16 token (k) tiles
NQ = T // 512     # 4 q-column tiles
LAG = 8           # PV runs this many steps behind S/exp
F32 = mybir.dt.float32
F32R = mybir.dt.float32r

_BUILT = None
LAST_RESULT = None


def _build():
    nc = bacc.Bacc(None, target_bir_lowering=False)

    xT = nc.dram_tensor("xT", [C, T], F32R, kind="ExternalInput")
    wqk = nc.dram_tensor("wqk", [C, 512], F32R, kind="ExternalInput")
    wv = nc.dram_tensor("wv", [C, 256], F32R, kind="ExternalInput")
    bqk = nc.dram_tensor("bqk", [4, 128], F32, kind="ExternalInput")
    # bv row: cols [0,256) = v bias, cols [256,384) = 1.0 (ones for bias matmul)
    bv = nc.dram_tensor("bv", [1, 512], F32R, kind="ExternalInput")
    y = nc.dram_tensor("y", [T, 256], F32, kind="ExternalOutput")

    with tile.TileContext(nc) as tc:
        ex = tc.nc  # alias
        with tc.tile_pool(name="persist", bufs=1) as sb:
            bqk_sb = sb.tile([128, 4], F32)
            bv_sb = sb.tile([1, 512], F32R)
            ones_col = sb.tile([128, 1], F32)
            ident = sb.tile([128, 128], F32)
            qkT = sb.tile([128, 4, T], F32R)
            v_aug = sb.tile([128, NT_T, HPC * 65], F32R)
            y_sb = sb.tile([128, NT_T, 256], F32)

            with tc.tile_pool(name="io", bufs=1) as io:
                xT_sb = io.tile([128, NT_C, T], F32R)
                wqk_sb = io.tile([128, NT_C, 512], F32R)
                wv_sb = io.tile([128, NT_C, 256], F32R)

                for ct in range(NT_C):
                    nc.sync.dma_start(out=wqk_sb[:, ct, :], in_=wqk[128 * ct:128 * (ct + 1), :])
                    nc.sync.dma_start(out=wv_sb[:, ct, :], in_=wv[128 * ct:128 * (ct + 1), :])
                    nc.sync.dma_start(out=xT_sb[:, ct, :], in_=xT[128 * ct:128 * (ct + 1), :])
                for ot in range(4):
                    nc.sync.dma_start(out=bqk_sb[:, ot:ot + 1],
                                      in_=bqk[ot:ot + 1, :].rearrange("o p -> p o"))
                nc.sync.dma_start(out=bv_sb[:, :], in_=bv[:, :])
                nc.vector.memset(ones_col[:, :], 1.0)
                masks.make_identity(nc, ident[:, :])
                nc.vector.tensor_copy(
                    v_aug.rearrange("p k (h e) -> p k h e", e=65)[:, :, :, 64:65],
                    ones_col[:, None, None, :].broadcast_to([128, NT_T, HPC, 1]),
                )

                # ---- phase A: head-pair-0 qk projection (own PSUM pool) ----
                with tc.tile_pool(name="ps_proj", bufs=4, space="PSUM") as psp:
                    for ot in (0, 2):
                        for tt in range(NQ):
                            ps = psp.tile([128, 512], F32, tag="psqk", name="ps")
                            for ct in range(NT_C):
                                nc.tensor.matmul(
                                    ps[:, :],
                                    wqk_sb[:, ct, 128 * ot:128 * (ot + 1)],
                                    xT_sb[:, ct, 512 * tt:512 * (tt + 1)],
                                    start=(ct == 0), stop=(ct == NT_C - 1),
                                )
                            nc.vector.tensor_scalar_add(
                                qkT[:, ot, 512 * tt:512 * (tt + 1)], ps[:, :],
                                bqk_sb[:, ot:ot + 1],
                            )

                # ---- phase B pools: pss 4 + pso 2 + misc 2 = 8 banks ----
                with tc.tile_pool(name="ps_s", bufs=2, space="PSUM") as pss, \
                     tc.tile_pool(name="ps_o", bufs=1, space="PSUM") as pso, \
                     tc.tile_pool(name="ps_m", bufs=2, space="PSUM") as psm, \
                     tc.tile_pool(name="esb", bufs=10) as esb, \
                     tc.tile_pool(name="small", bufs=4) as smb:

                    def v_proj(tt):
                        psv = psm.tile([128, 256], F32, tag="m", name="psv")
                        for ct in range(NT_C):
                            nc.tensor.matmul(
                                psv[:, :],
                                xT_sb[:, ct, 128 * tt:128 * (tt + 1)],
                                wv_sb[:, ct, :],
                                start=(ct == 0), stop=False,
                            )
                        nc.tensor.matmul(
                            psv[:, :], bv_sb[:, 256:384], bv_sb[:, 0:256],
                            start=False, stop=True,
                        )
                        nc.vector.tensor_copy(
                            v_aug.rearrange("p k (h e) -> p k h e", e=65)[:, tt, :, 0:64],
                            psv.rearrange("p (h e) -> p h e", e=64)[:, :, :],
                        )

                    def qk_proj_b(ot, tt):
                        # pair-1 qk projection, interleaved into phase B PE idle
                        ps = psm.tile([128, 512], F32, tag="m", name="psb")
                        for ct in range(NT_C):
                            nc.tensor.matmul(
                                ps[:, :],
                                wqk_sb[:, ct, 128 * ot:128 * (ot + 1)],
                                xT_sb[:, ct, 512 * tt:512 * (tt + 1)],
                                start=(ct == 0), stop=(ct == NT_C - 1),
                            )
                        nc.vector.tensor_scalar_add(
                            qkT[:, ot, 512 * tt:512 * (tt + 1)], ps[:, :],
                            bqk_sb[:, ot:ot + 1],
                        )

                    E = [None] * 128
                    po_cur = [None]

                    def s_task(i):
                        u, kt = divmod(i, NT_T)
                        pr, qt = divmod(u, NQ)
                        ps2 = pss.tile([128, 1024], F32, tag="s", name="ps2")
                        for h2 in range(2):
                            pb = 64 * h2
                            nc.tensor.matmul(
                                ps2[:, 512 * h2:512 * (h2 + 1)],
                                qkT[pb:pb + 64, 2 + pr, 128 * kt:128 * (kt + 1)],
                                qkT[pb:pb + 64, pr, 512 * qt:512 * (qt + 1)],
                                start=True, stop=True,
                            )
                        e = esb.tile([128, 1024], F32R, tag="e", name="e")
                        nc.scalar.activation(e[:, :], ps2[:, :],
                                             mybir.ActivationFunctionType.Exp)
                        E[i] = e

                    def pv_task(j):
                        u, kt = divmod(j, NT_T)
                        pr, qt = divmod(u, NQ)
                        if kt == 0:
                            po_cur[0] = pso.tile([65, 1024], F32, tag="po", name="po")
                        po = po_cur[0]
                        for h2 in range(2):
                            h = 2 * pr + h2
                            nc.tensor.matmul(
                                po[:, 512 * h2:512 * (h2 + 1)],
                                v_aug[:, kt, 65 * h:65 * (h + 1)],
                                E[j][:, 512 * h2:512 * (h2 + 1)],
                                start=(kt == 0), stop=(kt == NT_T - 1),
                            )
                        E[j] = None
                        if kt == NT_T - 1:
                            finalize(u, po)

                    def finalize(u, po):
                        pr, qt = divmod(u, NQ)
                        o_sb = smb.tile([65, 1024], F32, tag="osb", name="o_sb", bufs=2)
                        nc.vector.tensor_copy(o_sb[:, :], po[:, :])
                        for h2 in range(2):
                            h = 2 * pr + h2
                            for sq in range(4):
                                qi = 4 * qt + sq
                                pt = psm.tile([128, 65], F32, tag="m", name="pt")
                                nc.tensor.transpose(
                                    pt[:, :],
                                    o_sb[:, 512 * h2 + 128 * sq:512 * h2 + 128 * (sq + 1)],
                                    ident[0:65, 0:65],
                                )
                                rec = smb.tile([128, 1], F32, tag="rec", name="rec")
                                nc.vector.reciprocal(rec[:, :], pt[:, 64:65])
                                nc.vector.tensor_scalar_mul(
                                    y_sb[:, qi, 64 * h:64 * (h + 1)],
                                    pt[:, 0:64],
                                    rec[:, :],
                                )
                        if pr == 1:
                            for sq in range(4):
                                qi = 4 * qt + sq
                                nc.sync.dma_start(out=y[128 * qi:128 * (qi + 1), :],
                                                  in_=y_sb[:, qi, :])

                    # groups of 4 steps; PV runs LAG steps behind; pair-1 qk
                    # proj chains fill PE idle in groups 4..7 (units pr=0).
                    qk1 = [(ot, tt) for ot in (1, 3) for tt in range(NQ)]
                    for g in range(32):
                        if g < 4:
                            for tt in range(4 * g, 4 * g + 4):
                                v_proj(tt)
                        elif g < 8:
                            for ot, tt in qk1[2 * (g - 4):2 * (g - 4) + 2]:
                                qk_proj_b(ot, tt)
                        for i in range(4 * g, 4 * g + 4):
                            s_task(i)
                        for j in range(4 * g - LAG, 4 * g + 4 - LAG):
                            if j >= 0:
                                pv_task(j)
                    for j in range(128 - LAG, 128):
                        pv_task(j)

    nc.compile()
    return nc


def kernel(x, W_proj, b_proj):
    global _BUILT, LAST_RESULT
    x = np.ascontiguousarray(np.asarray(x, dtype=np.float32))
    W_proj = np.ascontiguousarray(np.asarray(W_proj, dtype=np.float32))
    b_proj = np.ascontiguousarray(np.asarray(b_proj, dtype=np.float32))

    if _BUILT is None:
        _BUILT = _build()
    nc = _BUILT

    in_maps = []
    for c in range(N_CORES):
        b, g = divmod(c, 4)
        hs = HPC * g                      # first global head of this core
        r0 = D * hs                       # first q row
        q_rows = W_proj[r0:r0 + 256] * 0.125
        k_rows = W_proj[C + r0:C + r0 + 256]
        v_rows = W_proj[2 * C + r0:2 * C + r0 + 256]
        in_maps.append({
            "xT": np.ascontiguousarray(x[b].T),
            "wqk": np.ascontiguousarray(np.concatenate([q_rows, k_rows], 0).T),
            "wv": np.ascontiguousarray(v_rows.T),
            "bqk": np.concatenate(
                [b_proj[r0:r0 + 256] * 0.125, b_proj[C + r0:C + r0 + 256]]
            ).reshape(4, 128).copy(),
            "bv": np.concatenate(
                [b_proj[2 * C + r0:2 * C + r0 + 256], np.ones(256, np.float32)]
            ).reshape(1, 512),
        })

    LAST_RESULT = run_bass_kernel_spmd(nc, in_maps, core_ids=list(range(N_CORES)))
    out = np.empty((B, T, C), dtype=np.float32)
    for c in range(N_CORES):
        b, g = divmod(c, 4)
        out[b, :, 256 * g:256 * (g + 1)] = LAST_RESULT.results[c]["y"]
    return out


# revision 16
# speedup vs baseline: 1.0365x; 1.0365x over previous
"""Multi-head self-attention (B=2, T=2048, C=1024, H=16) on 8 trn2 NeuronCores.

Sharding: tensor-parallel over heads x data-parallel over batch.
Core c handles batch b = c//4 and the 4 heads [4g, 4g+4) where g = c%4.

Per-core device kernel (matmuls in float32r = full-rate single-pass fp32):
  A. qkT proj (transposed layout, K=C contraction, bias added on DVE):
     qkT tiles [128, 2048]: tile 0=[qT_h0;qT_h1] 1=[qT_h2;qT_h3]
                            2=[kT_h0;kT_h1] 3=[kT_h2;kT_h3]  (q pre-scaled 1/8)
     v proj in natural layout [token, d], bias via ones-row matmul, stored
     ones-augmented: v_aug[t, 65h+d], col 65h+64 = 1.0.
  B. attention, 128 flat steps (unit u = head-pair pr x q-tile qt, k-tile kt):
     S^T pair: two K=64 matmuls (array row-halves pack) into one 2-bank PSUM
     tile [128, 1024]; one wide exp on ScalarE -> E tile (SBUF, f32r).
     P.V runs LAG=4 steps behind: o_aug[65, 1024] accumulates v_aug.T @ E
     over 16 k-tiles (row 64 = softmax denominator, from the ones column).
     Finalize per unit: PE-transpose 128-col chunks, DVE reciprocal + scale
     -> natural-layout y tile. ScalarE does nothing but exp; all copies/bias
     on DVE so the exp table never reloads.
"""

import numpy as np

import concourse.bass as bass
import concourse.mybir as mybir
import concourse.tile as tile
from concourse import bacc, masks
from concourse.bass_utils import run_bass_kernel_spmd

N_CORES = 8
B, T, C = 2, 2048, 1024
D = 64            # head dim
HPC = 4           # heads per core
NT_C = C // 128   # 8 contraction tiles
NT_T = T // 128   # 16 token (k) tiles
NQ = T // 512     # 4 q-column tiles
LAG = 8           # PV runs this many steps behind S/exp
F32 = mybir.dt.float32
F32R = mybir.dt.float32r

_BUILT = None
LAST_RESULT = None


def _build():
    nc = bacc.Bacc(None, target_bir_lowering=False)

    xT = nc.dram_tensor("xT", [C, T], F32R, kind="ExternalInput")
    wqk = nc.dram_tensor("wqk", [C, 512], F32R, kind="ExternalInput")
    wv = nc.dram_tensor("wv", [C, 256], F32R, kind="ExternalInput")
    bqk = nc.dram_tensor("bqk", [4, 128], F32, kind="ExternalInput")
    # bv row: cols [0,256) = v bias, cols [256,384) = 1.0 (ones for bias matmul)
    bv = nc.dram_tensor("bv", [1, 512], F32R, kind="ExternalInput")
    y = nc.dram_tensor("y", [T, 256], F32, kind="ExternalOutput")

    with tile.TileContext(nc) as tc:
        ex = tc.nc  # alias
        with tc.tile_pool(name="persist", bufs=1) as sb:
            bqk_sb = sb.tile([128, 4], F32)
            bv_sb = sb.tile([1, 512], F32R)
            ones_col = sb.tile([128, 1], F32)
            ident = sb.tile([128, 128], F32)
            qkT = sb.tile([128, 4, T], F32R)
            v_aug = sb.tile([128, NT_T, HPC * 65], F32R)
            y_sb = sb.tile([128, NT_T, 256], F32)

            with tc.tile_pool(name="io", bufs=1) as io:
                xT_sb = io.tile([128, NT_C, T], F32R)
                wqk_sb = io.tile([128, NT_C, 512], F32R)
                wv_sb = io.tile([128, NT_C, 256], F32R)

                for ct in range(NT_C):
                    nc.gpsimd.dma_start(out=wqk_sb[:, ct, :], in_=wqk[128 * ct:128 * (ct + 1), :])
                    nc.gpsimd.dma_start(out=wv_sb[:, ct, :], in_=wv[128 * ct:128 * (ct + 1), :])
                    nc.sync.dma_start(out=xT_sb[:, ct, :], in_=xT[128 * ct:128 * (ct + 1), :])
                for ot in range(4):
                    nc.sync.dma_start(out=bqk_sb[:, ot:ot + 1],
                                      in_=bqk[ot:ot + 1, :].rearrange("o p -> p o"))
                nc.sync.dma_start(out=bv_sb[:, :], in_=bv[:, :])
                nc.vector.memset(ones_col[:, :], 1.0)
                masks.make_identity(nc, ident[:, :])
                nc.vector.tensor_copy(
                    v_aug.rearrange("p k (h e) -> p k h e", e=65)[:, :, :, 64:65],
                    ones_col[:, None, None, :].broadcast_to([128, NT_T, HPC, 1]),
                )

                # ---- phase A: head-pair-0 qk projection (own PSUM pool) ----
                # tt-outer so S step 0 (needs ot0-tt0 + ot2-tt0) unblocks first
                with tc.tile_pool(name="ps_proj", bufs=4, space="PSUM") as psp:
                    for tt in range(NQ):
                        for ot in (0, 2):
                            ps = psp.tile([128, 512], F32, tag="psqk", name="ps")
                            for ct in range(NT_C):
                                nc.tensor.matmul(
                                    ps[:, :],
                                    wqk_sb[:, ct, 128 * ot:128 * (ot + 1)],
                                    xT_sb[:, ct, 512 * tt:512 * (tt + 1)],
                                    start=(ct == 0), stop=(ct == NT_C - 1),
                                )
                            nc.vector.tensor_scalar_add(
                                qkT[:, ot, 512 * tt:512 * (tt + 1)], ps[:, :],
                                bqk_sb[:, ot:ot + 1],
                            )

                # ---- phase B pools: pss 4 + pso 2 + misc 2 = 8 banks ----
                with tc.tile_pool(name="ps_s", bufs=2, space="PSUM") as pss, \
                     tc.tile_pool(name="ps_o", bufs=1, space="PSUM") as pso, \
                     tc.tile_pool(name="ps_m", bufs=2, space="PSUM") as psm, \
                     tc.tile_pool(name="esb", bufs=10) as esb, \
                     tc.tile_pool(name="small", bufs=4) as smb:

                    def v_proj(tt):
                        psv = psm.tile([128, 256], F32, tag="m", name="psv")
                        for ct in range(NT_C):
                            nc.tensor.matmul(
                                psv[:, :],
                                xT_sb[:, ct, 128 * tt:128 * (tt + 1)],
                                wv_sb[:, ct, :],
                                start=(ct == 0), stop=False,
                            )
                        nc.tensor.matmul(
                            psv[:, :], bv_sb[:, 256:384], bv_sb[:, 0:256],
                            start=False, stop=True,
                        )
                        nc.vector.tensor_copy(
                            v_aug.rearrange("p k (h e) -> p k h e", e=65)[:, tt, :, 0:64],
                            psv.rearrange("p (h e) -> p h e", e=64)[:, :, :],
                        )

                    def qk_proj_b(ot, tt):
                        # pair-1 qk projection, interleaved into phase B PE idle
                        ps = psm.tile([128, 512], F32, tag="m", name="psb")
                        for ct in range(NT_C):
                            nc.tensor.matmul(
                                ps[:, :],
                                wqk_sb[:, ct, 128 * ot:128 * (ot + 1)],
                                xT_sb[:, ct, 512 * tt:512 * (tt + 1)],
                                start=(ct == 0), stop=(ct == NT_C - 1),
                            )
                        nc.vector.tensor_scalar_add(
                            qkT[:, ot, 512 * tt:512 * (tt + 1)], ps[:, :],
                            bqk_sb[:, ot:ot + 1],
                        )

                    E = [None] * 128
                    po_cur = [None]

                    def s_task(i):
                        u, kt = divmod(i, NT_T)
                        pr, qt = divmod(u, NQ)
                        ps2 = pss.tile([128, 1024], F32, tag="s", name="ps2")
                        for h2 in range(2):
                            pb = 64 * h2
                            nc.tensor.matmul(
                                ps2[:, 512 * h2:512 * (h2 + 1)],
                                qkT[pb:pb + 64, 2 + pr, 128 * kt:128 * (kt + 1)],
                                qkT[pb:pb + 64, pr, 512 * qt:512 * (qt + 1)],
                                start=True, stop=True,
                            )
                        e = esb.tile([128, 1024], F32R, tag="e", name="e")
                        nc.scalar.activation(e[:, :], ps2[:, :],
                                             mybir.ActivationFunctionType.Exp)
                        E[i] = e

                    def pv_task(j):
                        u, kt = divmod(j, NT_T)
                        pr, qt = divmod(u, NQ)
                        if kt == 0:
                            po_cur[0] = pso.tile([65, 1024], F32, tag="po", name="po")
                        po = po_cur[0]
                        for h2 in range(2):
                            h = 2 * pr + h2
                            nc.tensor.matmul(
                                po[:, 512 * h2:512 * (h2 + 1)],
                                v_aug[:, kt, 65 * h:65 * (h + 1)],
                                E[j][:, 512 * h2:512 * (h2 + 1)],
                                start=(kt == 0), stop=(kt == NT_T - 1),
                            )
                        E[j] = None
                        if kt == NT_T - 1:
                            finalize(u, po)

                    def finalize(u, po):
                        pr, qt = divmod(u, NQ)
                        o_sb = smb.tile([65, 1024], F32, tag="osb", name="o_sb", bufs=2)
                        nc.vector.tensor_copy(o_sb[:, :], po[:, :])
                        for h2 in range(2):
                            h = 2 * pr + h2
                            for sq in range(4):
                                qi = 4 * qt + sq
                                pt = psm.tile([128, 65], F32, tag="m", name="pt")
                                nc.tensor.transpose(
                                    pt[:, :],
                                    o_sb[:, 512 * h2 + 128 * sq:512 * h2 + 128 * (sq + 1)],
                                    ident[0:65, 0:65],
                                )
                                rec = smb.tile([128, 1], F32, tag="rec", name="rec")
                                nc.vector.reciprocal(rec[:, :], pt[:, 64:65])
                                nc.vector.tensor_scalar_mul(
                                    y_sb[:, qi, 64 * h:64 * (h + 1)],
                                    pt[:, 0:64],
                                    rec[:, :],
                                )
                        if pr == 1:
                            for sq in range(4):
                                qi = 4 * qt + sq
                                nc.sync.dma_start(out=y[128 * qi:128 * (qi + 1), :],
                                                  in_=y_sb[:, qi, :])

                    # pairs of steps: emit both S pairs back-to-back (they fill
                    # both pss slots -> consecutive-pair discount on PE), then
                    # the lagged PV pair, then filler proj chains (v for the
                    # first 8 groups, pair-1 qk for the next 8).
                    qk1 = [(ot, tt) for tt in range(NQ) for ot in (1, 3)]
                    for gg in range(64):
                        i = 2 * gg
                        s_task(i)
                        s_task(i + 1)
                        for j in (i - LAG, i + 1 - LAG):
                            if j >= 0:
                                pv_task(j)
                        if gg < 8:
                            v_proj(2 * gg)
                            v_proj(2 * gg + 1)
                        elif gg < 16:
                            ot, tt = qk1[gg - 8]
                            qk_proj_b(ot, tt)
                    for j in range(128 - LAG, 128):
                        pv_task(j)

    nc.compile()
    return nc


def kernel(x, W_proj, b_proj):
    global _BUILT, LAST_RESULT
    x = np.ascontiguousarray(np.asarray(x, dtype=np.float32))
    W_proj = np.ascontiguousarray(np.asarray(W_proj, dtype=np.float32))
    b_proj = np.ascontiguousarray(np.asarray(b_proj, dtype=np.float32))

    if _BUILT is None:
        _BUILT = _build()
    nc = _BUILT

    in_maps = []
    for c in range(N_CORES):
        b, g = divmod(c, 4)
        hs = HPC * g                      # first global head of this core
        r0 = D * hs                       # first q row
        q_rows = W_proj[r0:r0 + 256] * 0.125
        k_rows = W_proj[C + r0:C + r0 + 256]
        v_rows = W_proj[2 * C + r0:2 * C + r0 + 256]
        in_maps.append({
            "xT": np.ascontiguousarray(x[b].T),
            "wqk": np.ascontiguousarray(np.concatenate([q_rows, k_rows], 0).T),
            "wv": np.ascontiguousarray(v_rows.T),
            "bqk": np.concatenate(
                [b_proj[r0:r0 + 256] * 0.125, b_proj[C + r0:C + r0 + 256]]
            ).reshape(4, 128).copy(),
            "bv": np.concatenate(
                [b_proj[2 * C + r0:2 * C + r0 + 256], np.ones(256, np.float32)]
            ).reshape(1, 512),
        })

    LAST_RESULT = run_bass_kernel_spmd(nc, in_maps, core_ids=list(range(N_CORES)))
    out = np.empty((B, T, C), dtype=np.float32)
    for c in range(N_CORES):
        b, g = divmod(c, 4)
        out[b, :, 256 * g:256 * (g + 1)] = LAST_RESULT.results[c]["y"]
    return out
